# revision 2
# baseline (speedup 1.0000x reference)
"""Trainium2 Bass kernel: ConAM-style patch attention (B,C,H,W)=(8,256,256,256), P=16.

out = x * (1 + att_up), att = softmax over 16x16 patch scores computed from a
tiny 2-layer MLP over per-patch means + a global mean feature.

Sharding: pure data parallel, one batch element per NeuronCore (8 cores).

v2 plan (channel-major unified layout, bf16 SBUF cache):
  Phase A: read x in (c=128, 16h*256w) tiles (16 KiB contiguous descriptors),
           DVE tensor_reduce(XY) per tile -> patch sums lf (c, 257), pw-major.
           The LAST n_cache tiles are also down-converted f32->bf16 by the
           scalar engine into persistent SBUF cache slots.
  Phase B: MLP on PE (weights pre-transposed on host, /256 folded into w1),
           softmax on 1 partition -> t1sb[ph, w] = att for h-patch row ph.
  Phase C: per h-patch row ph: build scB[c=128, w256] = 1 + att[ph, pw(w)]
           with one PE selector matmul; multiply tiles by scB broadcast along
           h (stride-0 view). Cached rows read bf16 from SBUF instead of HBM.
  Traffic: 64 MiB read + (64 - 2*n_cache) MiB read + 64 MiB write.
"""

import numpy as np

import concourse.bass as bass
import concourse.mybir as mybir
from concourse import bacc
from concourse.tile import TileContext
from concourse.bass_utils import run_bass_kernel_spmd

F32 = mybir.dt.float32
BF16 = mybir.dt.bfloat16
AF = mybir.ActivationFunctionType
ALU = mybir.AluOpType
AX = mybir.AxisListType

N_CORES = 8
C, H, W = 256, 256, 256
PS = 16  # patch size
NT = 32  # tiles per pass: 16 ph rows x 2 channel halves


def build_nc(reps=1, n_cache=16, work_bufs=3, dma_split=False,
             cache_dtype=BF16, scb_dtype=F32):
    nc = bacc.Bacc("TRN2", target_bir_lowering=False, debug=False)

    x = nc.dram_tensor("x", [C, H, W], F32, kind="ExternalInput")
    w1t = nc.dram_tensor("w1t", [C, C], F32, kind="ExternalInput")
    b1c = nc.dram_tensor("b1c", [C, 1], F32, kind="ExternalInput")
    w2t = nc.dram_tensor("w2t", [C, C], F32, kind="ExternalInput")
    b2c = nc.dram_tensor("b2c", [C, 1], F32, kind="ExternalInput")
    out = nc.dram_tensor("out", [C, H, W], F32, kind="ExternalOutput")

    # Inline 0/1 indicator constants (embedded in the NEFF).
    g16_np = np.zeros((16, 256), np.float32)
    for pw in range(16):
        g16_np[pw, pw * 16:(pw + 1) * 16] = 1.0
    # sel[q, ph*128 + o] = 1 iff q == ph: selects t1 row ph onto all 128
    # output partitions of a [16]x[16,128] matmul.
    sel_np = np.zeros((16, 16 * 128), np.float32)
    for ph in range(16):
        sel_np[ph, ph * 128:(ph + 1) * 128] = 1.0
    g16 = nc.inline_tensor(g16_np, "g16")
    sel = nc.inline_tensor(sel_np, "sel")

    ncached0 = NT - n_cache  # tile idx >= this is cached

    with TileContext(nc) as tc:
        with (
            tc.tile_pool(name="consts", bufs=1) as consts,
            tc.tile_pool(name="lfpool", bufs=1) as lfpool,
            tc.tile_pool(name="work", bufs=work_bufs) as work,
            tc.tile_pool(name="small", bufs=1) as small,
            tc.tile_pool(name="scbp", bufs=2) as scbp,
            tc.tile_pool(name="psum", bufs=1, space="PSUM") as psum,
            tc.tile_pool(name="psum2", bufs=2, space="PSUM") as psum2,
            tc.tile_pool(name="cachep", bufs=1) as cachep,
        ):
            # ---- constants to SBUF ------------------------------------
            w1s = consts.tile([128, 512], F32)  # [:, kt*256+o] rows=c-tile kt
            nc.sync.dma_start(out=w1s[:, 0:256], in_=w1t[0:128, :])
            nc.sync.dma_start(out=w1s[:, 256:512], in_=w1t[128:256, :])
            w2s = consts.tile([128, 512], F32)
            nc.sync.dma_start(out=w2s[:, 0:256], in_=w2t[0:128, :])
            nc.sync.dma_start(out=w2s[:, 256:512], in_=w2t[128:256, :])
            b1s = consts.tile([128, 2], F32)
            nc.sync.dma_start(out=b1s[:, 0:1], in_=b1c[0:128, :])
            nc.sync.dma_start(out=b1s[:, 1:2], in_=b1c[128:256, :])
            b2s = consts.tile([128, 2], F32)
            nc.sync.dma_start(out=b2s[:, 0:1], in_=b2c[0:128, :])
            nc.sync.dma_start(out=b2s[:, 1:2], in_=b2c[128:256, :])
            g16s = consts.tile([16, 256], F32)
            nc.sync.dma_start(out=g16s, in_=g16[:, :])
            sels = consts.tile([16, 16 * 128], F32)
            nc.sync.dma_start(out=sels, in_=sel[:, :])

            for _rep in range(reps):
                # ---- phase A: per-patch sums + bf16 cache fill --------
                # lf[c, n] with n = pw*16 + ph (pw-major); col 256 = global.
                lf0 = lfpool.tile([128, 257], F32, name="lf0", tag="lf0")
                lf1 = lfpool.tile([128, 257], F32, name="lf1", tag="lf1")
                lfs = [lf0, lf1]
                cach = {}
                for ph in range(16):
                    for ct in range(2):
                        tix = ph * 2 + ct
                        eng = nc.sync if (not dma_split or tix % 2 == 0) \
                            else nc.scalar
                        xt = work.tile([128, 16 * 256], F32, name="xt",
                                       tag="xt")
                        eng.dma_start(
                            out=xt.rearrange("p (h w) -> p h w", h=16),
                            in_=x[ct * 128:(ct + 1) * 128,
                                  ph * 16:(ph + 1) * 16, :],
                        )
                        dst = lfs[ct][:, 0:256].rearrange(
                            "p (pw q) -> p pw q", pw=16)[:, :, ph:ph + 1]
                        rview = xt.rearrange("p (h pw w) -> p pw h w",
                                             h=16, pw=16, w=16)
                        nc.vector.tensor_reduce(dst, rview, axis=AX.XY,
                                                op=ALU.add)
                        if tix >= ncached0:
                            ck = cachep.tile([128, 16 * 256], cache_dtype,
                                             name=f"ck{tix}", tag=f"ck{tix}")
                            nc.scalar.activation(ck, xt, AF.Copy)
                            cach[tix] = ck

                # ---- phase B: MLP + softmax ---------------------------
                for ct in range(2):
                    nc.vector.tensor_reduce(
                        lfs[ct][:, 256:257], lfs[ct][:, 0:256], axis=AX.X,
                        op=ALU.add)
                    nc.vector.tensor_scalar_mul(
                        lfs[ct][:, 256:257], lfs[ct][:, 256:257], 1.0 / 256.0)

                # layer 1: m1 = relu(w1 @ mix^T + b1); /256 folded into w1t.
                m1s = []
                for ot in range(2):
                    m1p = psum.tile([128, 257], F32, name=f"m1p{ot}",
                                    tag=f"m1p{ot}")
                    nc.tensor.matmul(m1p, w1s[:, ot * 128:(ot + 1) * 128],
                                     lf0, start=True, stop=False)
                    nc.tensor.matmul(
                        m1p, w1s[:, 256 + ot * 128:256 + (ot + 1) * 128],
                        lf1, start=False, stop=True)
                    m1t = small.tile([128, 257], F32, name=f"m1s{ot}",
                                     tag=f"m1s{ot}")
                    nc.scalar.activation(m1t, m1p, AF.Relu,
                                         bias=b1s[:, ot:ot + 1], scale=1.0)
                    m1s.append(m1t)

                # layer 2
                m2s = []
                for ot in range(2):
                    m2p = psum.tile([128, 257], F32, name=f"m2p{ot}",
                                    tag=f"m2p{ot}")
                    nc.tensor.matmul(m2p, w2s[:, ot * 128:(ot + 1) * 128],
                                     m1s[0], start=True, stop=False)
                    nc.tensor.matmul(
                        m2p, w2s[:, 256 + ot * 128:256 + (ot + 1) * 128],
                        m1s[1], start=False, stop=True)
                    m2t = small.tile([128, 257], F32, name=f"m2s{ot}",
                                     tag=f"m2s{ot}")
                    nc.scalar.activation(m2t, m2p, AF.Relu,
                                         bias=b2s[:, ot:ot + 1], scale=1.0)
                    m2s.append(m2t)

                # scores[n] = sum_c m2[c, n] * m2[c, 256]
                sp = psum.tile([1, 257], F32, name="sp", tag="sp")
                nc.tensor.matmul(sp, m2s[0][:, 256:257], m2s[0],
                                 start=True, stop=False)
                nc.tensor.matmul(sp, m2s[1][:, 256:257], m2s[1],
                                 start=False, stop=True)

                # softmax over the 256 patch scores (partition 0)
                negmax = small.tile([1, 1], F32)
                nc.vector.tensor_reduce(negmax, sp[0:1, 0:256], axis=AX.X,
                                        op=ALU.max, negate=True)
                exps = small.tile([1, 256], F32)
                nc.scalar.activation(exps, sp[0:1, 0:256], AF.Exp,
                                     bias=negmax, scale=1.0)
                ssum = small.tile([1, 1], F32)
                nc.vector.tensor_reduce(ssum, exps, axis=AX.X, op=ALU.add)
                rinv = small.tile([1, 1], F32)
                nc.vector.reciprocal(rinv, ssum)
                att = small.tile([1, 256], F32)
                nc.vector.tensor_scalar_mul(att, exps, rinv)

                # att (pw-major) -> attT[pw, ph] via reshape DMA
                attT = small.tile([16, 16], F32)
                nc.sync.dma_start(
                    out=attT, in_=att.rearrange("p (pw q) -> p pw q", pw=16))

                # t1[ph, w] = att[ph, w//16]
                t1p = psum.tile([16, 256], F32, name="t1p", tag="t1p")
                nc.tensor.matmul(t1p, attT, g16s, start=True, stop=True)
                t1sb = small.tile([16, 256], F32, name="t1sb", tag="t1sb")
                nc.scalar.activation(t1sb, t1p, AF.Copy)

                # ---- phase C: out = x * (1 + att) ---------------------
                for ph in range(16):
                    # scB[c, w] = 1 + t1[ph, w] on all 128 partitions
                    sbp = psum2.tile([128, 256], F32, name="scBp", tag="scBp")
                    nc.tensor.matmul(sbp, sels[:, ph * 128:(ph + 1) * 128],
                                     t1sb, start=True, stop=True)
                    scB = scbp.tile([128, 256], scb_dtype, name="scB",
                                    tag="scB")
                    nc.scalar.activation(scB, sbp, AF.Copy, bias=1.0)
                    scB_b = scB.unsqueeze(1).broadcast_to([128, 16, 256])
                    for ct in range(2):
                        tix = ph * 2 + ct
                        ld_eng = nc.sync if (not dma_split or tix % 2 == 0) \
                            else nc.scalar
                        st_eng = nc.sync if (not dma_split or tix % 2 == 1) \
                            else nc.scalar
                        wt = work.tile([128, 16 * 256], F32, name="xt",
                                       tag="xt")
                        wv = wt.rearrange("p (h w) -> p h w", h=16)
                        if tix in cach:
                            cv = cach[tix].rearrange("p (h w) -> p h w", h=16)
                            nc.vector.tensor_mul(wv, cv, scB_b)
                        else:
                            ld_eng.dma_start(
                                out=wv,
                                in_=x[ct * 128:(ct + 1) * 128,
                                      ph * 16:(ph + 1) * 16, :])
                            nc.vector.tensor_mul(wv, wv, scB_b)
                        st_eng.dma_start(
                            out=out[ct * 128:(ct + 1) * 128,
                                    ph * 16:(ph + 1) * 16, :],
                            in_=wv)

    nc.compile()
    return nc


_CACHE = {}


def _get_nc(reps=1, **kw):
    key = ("nc", reps, tuple(sorted(kw.items())))
    if key not in _CACHE:
        _CACHE[key] = build_nc(reps, **kw)
    return _CACHE[key]


def make_in_maps(x, w1, b1, w2, b2):
    x = np.ascontiguousarray(np.asarray(x, dtype=np.float32))
    w1 = np.asarray(w1, dtype=np.float32)
    b1 = np.asarray(b1, dtype=np.float32)
    w2 = np.asarray(w2, dtype=np.float32)
    b2 = np.asarray(b2, dtype=np.float32)
    w1t = np.ascontiguousarray(w1.T) * np.float32(1.0 / 256.0)
    w2t = np.ascontiguousarray(w2.T)
    b1c = np.ascontiguousarray(b1.reshape(C, 1))
    b2c = np.ascontiguousarray(b2.reshape(C, 1))
    return [
        {"x": x[i], "w1t": w1t, "b1c": b1c, "w2t": w2t, "b2c": b2c}
        for i in range(N_CORES)
    ]


def kernel(x, w1, b1, w2, b2):
    nc = _get_nc()
    in_maps = make_in_maps(x, w1, b1, w2, b2)
    res = run_bass_kernel_spmd(nc, in_maps, list(range(N_CORES))).results
    return np.stack([res[i]["out"] for i in range(N_CORES)], axis=0)
